# revision 14
# baseline (speedup 1.0000x reference)
"""Distributed Bass kernel: fused multi-head attention block on 8 TRN2 NeuronCores.

Problem: x[2,2048,1024] -> QKV proj -> RoPE(q,k) -> softmax(q k^T/8) v -> out proj.

Sharding: tensor-parallel over heads. 16 heads / 8 cores = 2 heads per core.
Each core computes QKV for its 2 heads (full sequence), RoPE, attention, then
chunked AllToAlls (4 pieces of 1024 tokens, pipelined under attention) convert
head-sharding to token-sharding so the output projection runs against the FULL
Wout with no AllReduce. Token ownership is interleaved per 128-token tile:
within piece p (tokens [p*1024,(p+1)*1024)), core j owns tokens
[p*1024+j*128, p*1024+(j+1)*128). Host reassembles the 4x128-row tiles.

Engine budget (vs v1 baseline at 331us):
 - all simple DMAs issue on the SYNC engine's hardware DGE (v1 put them on
   gpsimd software DGE at ~666ns dispatch each);
 - QKV PSUM->SBUF copies run on the otherwise-idle scalar(ACT) engine;
 - v tiles are transposed by XBAR DMA-transpose (v1 used PE transposes);
 - the AllToAll + out-proj + output DMA tail (~83us serial in v1) is chunked
   into 4 pieces and overlapped under the attention phase.

All layouts are pre-arranged on the host so the device never transposes x:
 - xT      [1024, 4096]  x^T               (shared by all cores)
 - wqkvT   [1024, 384]   [qA qB kA kB vA vB] rows of Wqkv, transposed (per core)
 - woutT   [1024, 1024]  Wout^T            (shared)
 - cos2/sin2 [128, 4096] RoPE factors expanded to d-major, two head copies
 - perm    [128, 128]    pair-swap permutation (rope partner via PE matmul)

Compute dtype bf16 (PE 1 cycle/row), f32 PSUM accumulation. Softmax skips the
max-subtraction (scores ~N(0,2), |s|<~12, exp safe in f32) and folds the
denominator into the PV matmul via a ones-column in the per-head v table
([key,130] slots: vA(64) | 1 | vB(64) | 1).
"""

import sys

for _p in ("/opt/trn_rl_repo", "/root/.axon_site/_ro/trn_rl_repo"):
    if _p not in sys.path:
        sys.path.append(_p)

import numpy as np
import ml_dtypes

B, N, HID = 2, 2048, 1024
H, DH = 16, 64
NCORES = 8
HPC = H // NCORES          # heads per core = 2
T = B * N                  # 4096 flattened tokens
TS = T // NCORES           # 512 tokens per core after AllToAll
EPC = HPC * DH             # 128 features per core
CH = 512                   # token chunk for QKV phase
NCH = T // CH              # 8 chunks
KT = 128                   # key tile
QC = 512                   # query chunk in attention
NPIECE = 4                 # a2a pieces (1024 tokens each)
PT = T // NPIECE           # 1024 tokens per piece
VS = 2 * (DH + 1)          # 130-wide v-table slot: [vA(64) | 1 | vB(64) | 1]

_bf16 = ml_dtypes.bfloat16


def _build_graph():
    import concourse.bass as bass
    import concourse.mybir as mybir
    import concourse.tile as tile
    from concourse import bacc

    f32 = mybir.dt.float32
    bf16 = mybir.dt.bfloat16

    nc = bacc.Bacc("TRN2", target_bir_lowering=False, debug=False, num_devices=NCORES)

    xT_e = nc.declare_dram_parameter("xT", [HID, T], bf16, isOutput=False)
    wqkvT_e = nc.declare_dram_parameter("wqkvT", [HID, 3 * EPC], bf16, isOutput=False)
    woutT_e = nc.declare_dram_parameter("woutT", [HID, HID], bf16, isOutput=False)
    cos2_e = nc.declare_dram_parameter("cos2", [2 * DH, T], bf16, isOutput=False)
    sin2_e = nc.declare_dram_parameter("sin2", [2 * DH, T], bf16, isOutput=False)
    perm_e = nc.declare_dram_parameter("perm", [128, 128], bf16, isOutput=False)
    ident_e = nc.declare_dram_parameter("ident", [128, 128], bf16, isOutput=False)
    out_e = nc.declare_dram_parameter("out", [TS, HID], f32, isOutput=True)

    with tile.TileContext(nc) as tc:
        with (
            tc.tile_pool(name="const", bufs=1) as cpool,
            tc.tile_pool(name="work", bufs=1) as wpool,
            tc.tile_pool(name="stream", bufs=4) as spool,
            tc.tile_pool(name="psum", bufs=2, space="PSUM") as pspool,
            tc.tile_pool(name="dram", bufs=1, space="DRAM") as dpool,
        ):
            # ---- constants / weights (gpsimd software DGE: async across queues) ----
            wqkvT = cpool.tile([128, 8 * 3 * EPC], bf16)       # 8 k-tiles side by side
            for kt in range(8):
                nc.gpsimd.dma_start(
                    wqkvT[:, kt * 3 * EPC:(kt + 1) * 3 * EPC],
                    wqkvT_e[kt * 128:(kt + 1) * 128, :],
                )
            perm = cpool.tile([128, 128], bf16)
            nc.gpsimd.dma_start(perm[:, :], perm_e[:, :])
            ident = cpool.tile([128, 128], bf16)
            nc.gpsimd.dma_start(ident[:, :], ident_e[:, :])
            woutT = cpool.tile([128, 8 * HID], bf16)
            cos2 = cpool.tile([128, T], bf16)
            sin2 = cpool.tile([128, T], bf16)

            # ---- persistent working tensors ----
            # HAM note: the PE clock-gate only releases (2.4 GHz) for
            # full-geometry matmuls. Scores therefore contract over the full
            # 128 partitions (both heads' d stacked) against ZERO-PADDED
            # per-head q copies. PV contracts over 128 keys with a 65-wide
            # per-head v table [v(64) | ones(1)] so the softmax denominator
            # rides along as output row 64.
            q_sb = wpool.tile([128, T], bf16)      # raw q (rope intermediate)
            k_sb = wpool.tile([128, T], bf16)      # becomes roped k
            qzA = wpool.tile([128, T], bf16)       # roped qA rows 0-63, 0 below
            qzB = wpool.tile([128, T], bf16)       # roped qB rows 64-127, 0 above
            vtab = wpool.tile([128, 32 * VS], bf16)  # [key, vA|1|vB|1] per slot
            ovT = wpool.tile([128, T], bf16)       # attention out ^T

            vt3 = vtab.rearrange("p (s c) -> p s c", c=VS)
            nc.vector.memset(qzA[DH:128, :], 0.0)
            nc.vector.memset(qzB[0:DH, :], 0.0)
            nc.vector.memset(vt3[:, :, DH:DH + 1], 1.0)
            nc.vector.memset(vt3[:, :, 2 * DH + 1:2 * DH + 2], 1.0)

            # ================= Phase 1: QKV + RoPE + v-transpose =================
            for c in range(NCH):
                xs = []
                for kt in range(8):
                    xt = spool.tile([128, CH], bf16, tag="xs", bufs=10)
                    nc.gpsimd.dma_start(
                        xt[:, :], xT_e[kt * 128:(kt + 1) * 128, c * CH:(c + 1) * CH]
                    )
                    xs.append(xt)
                if c < 2:
                    # rope factors for batch-half c arrive under chunk-c compute
                    hs = slice(c * (T // 2), (c + 1) * (T // 2))
                    nc.gpsimd.dma_start(sin2[:, hs], sin2_e[:, hs])
                    nc.gpsimd.dma_start(cos2[:, hs], cos2_e[:, hs])
                sl = slice(c * CH, (c + 1) * CH)
                # q, k, v: full-width matmuls, PSUM->SBUF copy on vector engine
                vTc = spool.tile([128, CH], bf16, tag="vTc", bufs=2)
                for which, dest, dsl in ((0, q_sb, sl), (1, k_sb, sl),
                                         (2, vTc, slice(0, CH))):
                    ps = pspool.tile([128, CH], f32, tag="mm", bufs=2)
                    for kt in range(8):
                        nc.tensor.matmul(
                            ps[:, :],
                            wqkvT[:, kt * 3 * EPC + which * EPC:
                                  kt * 3 * EPC + (which + 1) * EPC],
                            xs[kt][:, :],
                            start=(kt == 0),
                            stop=(kt == 7),
                        )
                    nc.vector.tensor_copy(dest[:, dsl], ps[:, :])
                # transpose v chunk into the 130-wide per-slot v tables
                for tt in range(CH // 128):
                    slot = c * (CH // 128) + tt
                    tsl = slice(tt * 128, (tt + 1) * 128)
                    tp = pspool.tile([128, 128], bf16, tag="sc", bufs=2)
                    nc.tensor.transpose(tp[:, :], vTc[:, tsl], ident[:, :])
                    nc.vector.tensor_copy(vt3[:, slot, 0:DH], tp[:, 0:DH])
                    nc.vector.tensor_copy(
                        vt3[:, slot, DH + 1:2 * DH + 1], tp[:, DH:2 * DH]
                    )

                # RoPE on q and k: t = P@x * sin2 ; rot = x*cos2 + t
                # k rotates in place; q writes split per head into qzA/qzB.
                for srd, dests in ((q_sb, ((qzA, 0, DH), (qzB, DH, 128))),
                                   (k_sb, ((k_sb, 0, 128),))):
                    pps = pspool.tile([128, CH], f32, tag="mm", bufs=2)
                    nc.tensor.matmul(
                        pps[:, :], perm[:, :], srd[:, sl],
                        start=True, stop=True,
                    )
                    tmp = spool.tile([128, CH], bf16, tag="ropetmp", bufs=2)
                    nc.vector.tensor_mul(tmp[:, :], pps[:, :], sin2[:, sl])
                    nc.vector.tensor_mul(srd[:, sl], srd[:, sl], cos2[:, sl])
                    for dst, p0, p1 in dests:
                        nc.vector.tensor_add(
                            dst[p0:p1, sl], srd[p0:p1, sl], tmp[p0:p1, :]
                        )

            # ================= Phase 2: attention per (batch, head) =================
            # Zippered software pipeline: the scores+exp of query-chunk i are
            # interleaved pairwise with the PV matmuls of chunk i-1 so the PE
            # stream stays dense while ACT churns through the exps. A2A pieces
            # and out-proj tiles are woven between chunks so the collective and
            # the output projection run under the attention compute.
            NKT = N // KT                      # 16 key tiles per chunk
            qzs = (qzA, qzB)

            def emit_pv_pair(st, pair):
                (b, qc, ops, expTs) = st
                for h in range(HPC):
                    for kt in (2 * pair, 2 * pair + 1):
                        slot = b * (N // 128) + kt
                        nc.tensor.matmul(
                            ops[0:DH + 1, h * QC:(h + 1) * QC],
                            vtab[:, slot * VS + h * (DH + 1):
                                 slot * VS + (h + 1) * (DH + 1)],
                            expTs[h][:, kt * QC:(kt + 1) * QC],
                            start=(kt == 0),
                            stop=(kt == NKT - 1),
                        )

            def emit_normalize(st):
                (b, qc, ops, expTs) = st
                q0 = b * N + qc * QC
                for h in range(HPC):
                    hr = h * DH
                    hc = h * QC
                    den = spool.tile([1, QC], f32, tag="den", bufs=2)
                    nc.vector.tensor_copy(den[0:1, :], ops[DH:DH + 1, hc:hc + QC])
                    rec = spool.tile([1, QC], f32, tag="rec", bufs=2)
                    nc.vector.reciprocal_approx_fast(rec[0:1, :], den[0:1, :])
                    bcs = spool.tile([64, QC], f32, tag="bcs", bufs=2)
                    nc.gpsimd.partition_broadcast(bcs[:, :], rec[0:1, :])
                    nc.vector.tensor_mul(
                        ovT[hr:hr + DH, q0:q0 + QC], ops[0:DH, hc:hc + QC], bcs[:, :]
                    )

            a2a_in = []
            a2a_out = []
            for p in range(NPIECE):
                a2a_in.append(dpool.tile([NCORES * 128, PT // NCORES], bf16,
                                         name=f"a2a_in{p}"))
                a2a_out.append(dpool.tile([NCORES * 128, PT // NCORES], bf16,
                                          name=f"a2a_out{p}"))

            def emit_comm(p):
                # stage my features for every peer's token tile of piece p,
                # run the AllToAll, pull the 8 source blocks into SBUF
                for j in range(NCORES):
                    c0 = p * PT + j * 128
                    nc.gpsimd.dma_start(
                        a2a_in[p][j * 128:(j + 1) * 128, :],
                        ovT[:, c0:c0 + 128],
                    )
                nc.gpsimd.collective_compute(
                    "AllToAll",
                    mybir.AluOpType.bypass,
                    ins=[a2a_in[p].opt()],
                    outs=[a2a_out[p].opt()],
                    replica_groups=[list(range(NCORES))],
                )
                gT = spool.tile([128, NCORES * 128], bf16, tag="gT", bufs=2)
                for s in range(NCORES):
                    nc.gpsimd.dma_start(
                        gT[:, s * 128:(s + 1) * 128],
                        a2a_out[p][s * 128:(s + 1) * 128, :],
                    )
                return gT

            def emit_outproj(p, gT):
                for nn in range(HID // 512):
                    odps = pspool.tile([128, 512], f32, tag="mm", bufs=2)
                    for s in range(8):
                        nc.tensor.matmul(
                            odps[:, :],
                            gT[:, s * 128:(s + 1) * 128],
                            woutT[:, s * HID + nn * 512:s * HID + (nn + 1) * 512],
                            start=(s == 0),
                            stop=(s == 7),
                        )
                    osb = spool.tile([128, 512], f32, tag="osb", bufs=2)
                    nc.vector.tensor_copy(osb[:, :], odps[:, :])
                    nc.gpsimd.dma_start(
                        out_e[p * 128:(p + 1) * 128, nn * 512:(nn + 1) * 512],
                        osb[:, :],
                    )

            for kt in range(8):
                nc.gpsimd.dma_start(
                    woutT[:, kt * HID:(kt + 1) * HID],
                    woutT_e[kt * 128:(kt + 1) * 128, :],
                )

            gTs = {}
            pending = None
            for ci in range(B * (N // QC)):
                b, qc = divmod(ci, N // QC)
                q0 = b * N + qc * QC
                expTs = (spool.tile([128, NKT * QC], bf16, name="expTA",
                                    tag="expTA", bufs=2),
                         spool.tile([128, NKT * QC], bf16, name="expTB",
                                    tag="expTB", bufs=2))
                for pair in range(NKT // 2):
                    for h in range(HPC):
                        sps = pspool.tile([128, 2 * QC], f32, tag="sc",
                                          bufs=2)
                        for half in range(2):
                            k0 = b * N + (2 * pair + half) * KT
                            nc.tensor.matmul(
                                sps[:, half * QC:(half + 1) * QC],
                                k_sb[:, k0:k0 + KT],
                                qzs[h][:, q0:q0 + QC],
                                start=True, stop=True,
                            )
                        nc.scalar.activation(
                            expTs[h][:, 2 * pair * QC:(2 * pair + 2) * QC],
                            sps[:, :],
                            mybir.ActivationFunctionType.Exp,
                            scale=DH ** -0.5,
                        )
                    if pending is not None:
                        emit_pv_pair(pending, pair)
                        if pair == NKT // 2 - 1:
                            emit_normalize(pending)
                ops = pspool.tile([128, 2 * QC], f32, tag="pv", bufs=1)
                pending = (b, qc, ops, expTs)
                if ci in (2, 4, 6):
                    p = (ci - 2) // 2
                    gTs[p] = emit_comm(p)
                if ci in (4, 6):
                    p = (ci - 4) // 2
                    emit_outproj(p, gTs.pop(p))
            for pair in range(NKT // 2):
                emit_pv_pair(pending, pair)
            emit_normalize(pending)
            gTs[3] = emit_comm(3)
            emit_outproj(2, gTs.pop(2))
            emit_outproj(3, gTs.pop(3))

    nc.finalize()
    return nc


def _host_inputs(x, rope, Wqkv, Wout):
    """Build the 8 per-core input maps with host-side layout prep."""
    xf = np.ascontiguousarray(x.reshape(T, HID).T).astype(_bf16)        # [1024, 4096]
    woutT = np.ascontiguousarray(Wout.T).astype(_bf16)                  # [1024, 1024]

    rf = rope.reshape(T, DH)                                            # [4096, 64]
    cosE = np.repeat(rf[:, 0::2], 2, axis=1).T                          # [64, 4096]
    sinE = np.repeat(rf[:, 1::2], 2, axis=1).T
    sgn = np.where(np.arange(DH) % 2 == 0, -1.0, 1.0)[:, None]
    sinS = (sinE * sgn)
    cos2 = np.ascontiguousarray(np.concatenate([cosE, cosE], 0)).astype(_bf16)
    sin2 = np.ascontiguousarray(np.concatenate([sinS, sinS], 0)).astype(_bf16)

    pm = np.zeros((128, 128), np.float32)
    for d in range(128):
        pm[d ^ 1, d] = 1.0       # partner[d] = q[d^1]; lhsT = S (symmetric)
    perm = pm.astype(_bf16)
    ident = np.eye(128, dtype=np.float32).astype(_bf16)

    w3 = Wqkv.reshape(3, H, DH, HID)
    in_maps = []
    for c in range(NCORES):
        blocks = []
        for which in range(3):
            for hl in range(HPC):
                blocks.append(w3[which, 2 * c + hl])                    # [64, 1024]
        wq = np.concatenate(blocks, 0)                                  # [384, 1024]
        wqkvT = np.ascontiguousarray(wq.T).astype(_bf16)                # [1024, 384]
        in_maps.append({
            "xT": xf, "wqkvT": wqkvT, "woutT": woutT,
            "cos2": cos2, "sin2": sin2, "perm": perm, "ident": ident,
        })
    return in_maps


_CACHE = {}


def kernel(x, rope, Wqkv, Wout):
    from concourse.bass_utils import run_bass_kernel_spmd

    if "nc" not in _CACHE:
        _CACHE["nc"] = _build_graph()
    nc = _CACHE["nc"]
    in_maps = _host_inputs(np.asarray(x, np.float32), np.asarray(rope, np.float32),
                           np.asarray(Wqkv, np.float32), np.asarray(Wout, np.float32))
    res = run_bass_kernel_spmd(nc, in_maps, core_ids=list(range(NCORES)))
    parts = np.stack([np.asarray(res.results[i]["out"], np.float32)
                      for i in range(NCORES)])                          # [8, 512, 1024]
    # core j's rows [p*128:(p+1)*128] hold global tokens p*1024 + j*128 ..
    full = parts.reshape(NCORES, NPIECE, 128, HID).transpose(1, 0, 2, 3)
    return np.ascontiguousarray(full.reshape(T, HID)).reshape(B, N, HID)


# revision 16
# speedup vs baseline: 1.0545x; 1.0545x over previous
"""Distributed Bass kernel: fused multi-head attention block on 8 TRN2 NeuronCores.

Problem: x[2,2048,1024] -> QKV proj -> RoPE(q,k) -> softmax(q k^T/8) v -> out proj.

Sharding: tensor-parallel over heads. 16 heads / 8 cores = 2 heads per core.
Each core computes QKV for its 2 heads (full sequence), RoPE, attention, then
chunked AllToAlls (4 pieces of 1024 tokens, cc_dim=Free, pipelined under the
attention compute) convert head-sharding to token-sharding so the output
projection runs against the FULL Wout with no AllReduce. Token ownership is
interleaved per 128-token tile: within piece p core j owns tokens
[p*1024+j*128, p*1024+(j+1)*128). Host reassembles the 4x128-row tiles.

Schedule: QKV/RoPE for batch 0 runs first (chunks 0-3); then batch-1 QKV
chunks interleave with batch-0 attention chunks so the scalar engine's exp
stream starts ~50us earlier and the PE never idles long enough to re-throttle
(HAM). A dummy 2KB AllToAll during phase 1 absorbs the CC-stream first-op
warmup (~11.5us) off the critical path. Per-piece gathers are emitted two
chunks after the collective trigger so the gpsimd queue never parks on an
in-flight collective while attention still needs broadcasts.

Compute dtype bf16, f32 PSUM accumulation. Softmax skips the max-subtraction
(scores ~N(0,2), exp safe in f32) and folds the denominator into the PV matmul
via a ones-column in the per-head v table ([key,130] slots: vA|1|vB|1).
"""

import sys

for _p in ("/opt/trn_rl_repo", "/root/.axon_site/_ro/trn_rl_repo"):
    if _p not in sys.path:
        sys.path.append(_p)

import numpy as np
import ml_dtypes

B, N, HID = 2, 2048, 1024
H, DH = 16, 64
NCORES = 8
HPC = H // NCORES          # heads per core = 2
T = B * N                  # 4096 flattened tokens
TS = T // NCORES           # 512 tokens per core after AllToAll
EPC = HPC * DH             # 128 features per core
CH = 512                   # token chunk for QKV phase
NCH = T // CH              # 8 chunks
KT = 128                   # key tile
QC = 512                   # query chunk in attention
NPIECE = 4                 # a2a pieces (1024 tokens each)
PT = T // NPIECE           # 1024 tokens per piece
VS = 2 * (DH + 1)          # 130-wide v-table slot: [vA(64) | 1 | vB(64) | 1]

_bf16 = ml_dtypes.bfloat16


def _build_graph():
    import concourse.bass as bass
    import concourse.mybir as mybir
    import concourse.tile as tile
    from concourse import bacc

    f32 = mybir.dt.float32
    bf16 = mybir.dt.bfloat16

    nc = bacc.Bacc("TRN2", target_bir_lowering=False, debug=False, num_devices=NCORES)

    xT_e = nc.declare_dram_parameter("xT", [HID, T], bf16, isOutput=False)
    wqkvT_e = nc.declare_dram_parameter("wqkvT", [HID, 3 * EPC], bf16, isOutput=False)
    woutT_e = nc.declare_dram_parameter("woutT", [HID, HID], bf16, isOutput=False)
    cos2_e = nc.declare_dram_parameter("cos2", [2 * DH, T], bf16, isOutput=False)
    sin2_e = nc.declare_dram_parameter("sin2", [2 * DH, T], bf16, isOutput=False)
    perm_e = nc.declare_dram_parameter("perm", [128, 128], bf16, isOutput=False)
    ident_e = nc.declare_dram_parameter("ident", [128, 128], bf16, isOutput=False)
    out_e = nc.declare_dram_parameter("out", [TS, HID], f32, isOutput=True)

    with tile.TileContext(nc) as tc:
        with (
            tc.tile_pool(name="const", bufs=1) as cpool,
            tc.tile_pool(name="work", bufs=1) as wpool,
            tc.tile_pool(name="stream", bufs=4) as spool,
            tc.tile_pool(name="psum", bufs=2, space="PSUM") as pspool,
            tc.tile_pool(name="dram", bufs=1, space="DRAM") as dpool,
        ):
            # ---- constants / weights ----
            wqkvT = cpool.tile([128, 8 * 3 * EPC], bf16)       # 8 k-tiles side by side
            for kt in range(8):
                nc.gpsimd.dma_start(
                    wqkvT[:, kt * 3 * EPC:(kt + 1) * 3 * EPC],
                    wqkvT_e[kt * 128:(kt + 1) * 128, :],
                )
            perm = cpool.tile([128, 128], bf16)
            nc.gpsimd.dma_start(perm[:, :], perm_e[:, :])
            ident = cpool.tile([128, 128], bf16)
            nc.gpsimd.dma_start(ident[:, :], ident_e[:, :])
            woutT = cpool.tile([128, 8 * HID], bf16)
            cos2 = cpool.tile([128, T], bf16)
            sin2 = cpool.tile([128, T], bf16)

            # ---- persistent working tensors ----
            q_sb = wpool.tile([128, T], bf16)      # raw q (rope intermediate)
            k_sb = wpool.tile([128, T], bf16)      # becomes roped k
            qzA = wpool.tile([128, T], bf16)       # roped qA rows 0-63, 0 below
            qzB = wpool.tile([128, T], bf16)       # roped qB rows 64-127, 0 above
            vtab = wpool.tile([128, 32 * VS], bf16)  # [key, vA|1|vB|1] per slot
            ovT = wpool.tile([128, T], bf16)       # attention out ^T

            vt3 = vtab.rearrange("p (s c) -> p s c", c=VS)
            nc.vector.memset(qzA[DH:128, :], 0.0)
            nc.vector.memset(qzB[0:DH, :], 0.0)
            nc.vector.memset(vt3[:, :, DH:DH + 1], 1.0)
            nc.vector.memset(vt3[:, :, 2 * DH + 1:2 * DH + 2], 1.0)

            # ================= Phase 1 chunk: QKV + RoPE + v-transpose ============
            def phase1_chunk(c):
                xs = []
                for kt in range(8):
                    xt = spool.tile([128, CH], bf16, tag="xs", bufs=10)
                    nc.gpsimd.dma_start(
                        xt[:, :], xT_e[kt * 128:(kt + 1) * 128, c * CH:(c + 1) * CH]
                    )
                    xs.append(xt)
                if c < 2:
                    # rope factors for batch-half c arrive under chunk-c compute
                    hs = slice(c * (T // 2), (c + 1) * (T // 2))
                    nc.gpsimd.dma_start(sin2[:, hs], sin2_e[:, hs])
                    nc.gpsimd.dma_start(cos2[:, hs], cos2_e[:, hs])
                sl = slice(c * CH, (c + 1) * CH)
                vTc = spool.tile([128, CH], bf16, tag="vTc", bufs=2)
                for which, dest, dsl in ((0, q_sb, sl), (1, k_sb, sl),
                                         (2, vTc, slice(0, CH))):
                    ps = pspool.tile([128, CH], f32, tag="mm", bufs=2)
                    for kt in range(8):
                        nc.tensor.matmul(
                            ps[:, :],
                            wqkvT[:, kt * 3 * EPC + which * EPC:
                                  kt * 3 * EPC + (which + 1) * EPC],
                            xs[kt][:, :],
                            start=(kt == 0),
                            stop=(kt == 7),
                        )
                    nc.vector.tensor_copy(dest[:, dsl], ps[:, :])
                # transpose v chunk into the 130-wide per-slot v tables
                for tt in range(CH // 128):
                    slot = c * (CH // 128) + tt
                    tsl = slice(tt * 128, (tt + 1) * 128)
                    tp = pspool.tile([128, 128], bf16, tag="sc", bufs=2)
                    nc.tensor.transpose(tp[:, :], vTc[:, tsl], ident[:, :])
                    nc.vector.tensor_copy(vt3[:, slot, 0:DH], tp[:, 0:DH])
                    nc.vector.tensor_copy(
                        vt3[:, slot, DH + 1:2 * DH + 1], tp[:, DH:2 * DH]
                    )
                # RoPE on q and k: t = P@x * sin2 ; rot = x*cos2 + t
                for srd, dests in ((q_sb, ((qzA, 0, DH), (qzB, DH, 128))),
                                   (k_sb, ((k_sb, 0, 128),))):
                    pps = pspool.tile([128, CH], f32, tag="mm", bufs=2)
                    nc.tensor.matmul(
                        pps[:, :], perm[:, :], srd[:, sl],
                        start=True, stop=True,
                    )
                    tmp = spool.tile([128, CH], bf16, tag="ropetmp", bufs=2)
                    nc.vector.tensor_mul(tmp[:, :], pps[:, :], sin2[:, sl])
                    nc.vector.tensor_mul(srd[:, sl], srd[:, sl], cos2[:, sl])
                    for dst, p0, p1 in dests:
                        nc.vector.tensor_add(
                            dst[p0:p1, sl], srd[p0:p1, sl], tmp[p0:p1, :]
                        )

            # ================= Attention machinery ================================
            NKT = N // KT                      # 16 key tiles per chunk
            qzs = (qzA, qzB)

            def emit_pv_pair(st, pair):
                (b, qc, ops, expTs) = st
                for h in range(HPC):
                    for kt in (2 * pair, 2 * pair + 1):
                        slot = b * (N // 128) + kt
                        nc.tensor.matmul(
                            ops[0:DH + 1, h * QC:(h + 1) * QC],
                            vtab[:, slot * VS + h * (DH + 1):
                                 slot * VS + (h + 1) * (DH + 1)],
                            expTs[h][:, kt * QC:(kt + 1) * QC],
                            start=(kt == 0),
                            stop=(kt == NKT - 1),
                        )

            def emit_normalize(st):
                (b, qc, ops, expTs) = st
                q0 = b * N + qc * QC
                for h in range(HPC):
                    hr = h * DH
                    hc = h * QC
                    den = spool.tile([1, QC], f32, tag="den", bufs=2)
                    nc.vector.tensor_copy(den[0:1, :], ops[DH:DH + 1, hc:hc + QC])
                    rec = spool.tile([1, QC], f32, tag="rec", bufs=2)
                    nc.vector.reciprocal_approx_fast(rec[0:1, :], den[0:1, :])
                    bcs = spool.tile([64, QC], f32, tag="bcs", bufs=2)
                    nc.gpsimd.partition_broadcast(bcs[:, :], rec[0:1, :])
                    nc.vector.tensor_mul(
                        ovT[hr:hr + DH, q0:q0 + QC], ops[0:DH, hc:hc + QC], bcs[:, :]
                    )

            a2a_in = [dpool.tile([NCORES * 128, PT // NCORES], bf16,
                                 name=f"a2a_in{p}") for p in range(NPIECE)]
            a2a_out = [dpool.tile([NCORES * 128, PT // NCORES], bf16,
                                  name=f"a2a_out{p}") for p in range(NPIECE)]

            def emit_comm(p):
                # stage my features for every peer's token tile of piece p,
                # then AllToAll (input split along dim 0, one block per peer)
                for j in range(NCORES):
                    c0 = p * PT + j * 128
                    nc.gpsimd.dma_start(
                        a2a_in[p][j * 128:(j + 1) * 128, :],
                        ovT[:, c0:c0 + 128],
                    )
                nc.gpsimd.collective_compute(
                    "AllToAll",
                    mybir.AluOpType.bypass,
                    ins=[a2a_in[p].opt()],
                    outs=[a2a_out[p].opt()],
                    replica_groups=[list(range(NCORES))],
                )

            def emit_outproj(p):
                gT = spool.tile([128, NCORES * 128], bf16, tag="gT", bufs=2)
                for s in range(NCORES):
                    nc.gpsimd.dma_start(
                        gT[:, s * 128:(s + 1) * 128],
                        a2a_out[p][s * 128:(s + 1) * 128, :],
                    )
                for nn in range(HID // 512):
                    odps = pspool.tile([128, 512], f32, tag="mm", bufs=2)
                    for s in range(8):
                        nc.tensor.matmul(
                            odps[:, :],
                            gT[:, s * 128:(s + 1) * 128],
                            woutT[:, s * HID + nn * 512:s * HID + (nn + 1) * 512],
                            start=(s == 0),
                            stop=(s == 7),
                        )
                    osb = spool.tile([128, 512], f32, tag="osb", bufs=2)
                    nc.vector.tensor_copy(osb[:, :], odps[:, :])
                    nc.gpsimd.dma_start(
                        out_e[p * 128:(p + 1) * 128, nn * 512:(nn + 1) * 512],
                        osb[:, :],
                    )

            st = {"pending": None}

            def attn_chunk(ci):
                b, qc = divmod(ci, N // QC)
                q0 = b * N + qc * QC
                expTs = (spool.tile([128, NKT * QC], bf16, name="expTA",
                                    tag="expTA", bufs=2),
                         spool.tile([128, NKT * QC], bf16, name="expTB",
                                    tag="expTB", bufs=2))
                for pair in range(NKT // 2):
                    for h in range(HPC):
                        sps = pspool.tile([128, 2 * QC], f32, tag="sc", bufs=2)
                        for half in range(2):
                            k0 = b * N + (2 * pair + half) * KT
                            nc.tensor.matmul(
                                sps[:, half * QC:(half + 1) * QC],
                                k_sb[:, k0:k0 + KT],
                                qzs[h][:, q0:q0 + QC],
                                start=True, stop=True,
                            )
                        nc.scalar.activation(
                            expTs[h][:, 2 * pair * QC:(2 * pair + 2) * QC],
                            sps[:, :],
                            mybir.ActivationFunctionType.Exp,
                            scale=DH ** -0.5,
                        )
                    if st["pending"] is not None:
                        emit_pv_pair(st["pending"], pair)
                        if pair == NKT // 2 - 1:
                            emit_normalize(st["pending"])
                ops = pspool.tile([128, 2 * QC], f32, tag="pv", bufs=1)
                st["pending"] = (b, qc, ops, expTs)
                if ci in (2, 4, 6):
                    emit_comm((ci - 2) // 2)
                if ci in (4, 6):
                    emit_outproj((ci - 4) // 2)

            # ================= Top-level schedule ================================
            # CC-stream warmup: tiny collective with no dependencies, absorbed
            # under phase-1 compute (its payload is never read).
            ccw_in = dpool.tile([NCORES, 128], bf16, name="ccw_in")
            ccw_out = dpool.tile([NCORES, 128], bf16, name="ccw_out")

            for c in range(4):
                phase1_chunk(c)
                if c == 0:
                    nc.gpsimd.collective_compute(
                        "AllToAll",
                        mybir.AluOpType.bypass,
                        ins=[ccw_in.opt()],
                        outs=[ccw_out.opt()],
                        replica_groups=[list(range(NCORES))],
                    )
            for kt in range(8):
                nc.gpsimd.dma_start(
                    woutT[:, kt * HID:(kt + 1) * HID],
                    woutT_e[kt * 128:(kt + 1) * 128, :],
                )
            for i in range(4):
                phase1_chunk(4 + i)
                attn_chunk(i)
            for ci in range(4, 8):
                attn_chunk(ci)
            for pair in range(NKT // 2):
                emit_pv_pair(st["pending"], pair)
            emit_normalize(st["pending"])
            emit_comm(3)
            emit_outproj(2)
            emit_outproj(3)

    nc.finalize()
    return nc


def _host_inputs(x, rope, Wqkv, Wout):
    """Build the 8 per-core input maps with host-side layout prep."""
    xf = np.ascontiguousarray(x.reshape(T, HID).T).astype(_bf16)        # [1024, 4096]
    woutT = np.ascontiguousarray(Wout.T).astype(_bf16)                  # [1024, 1024]

    rf = rope.reshape(T, DH)                                            # [4096, 64]
    cosE = np.repeat(rf[:, 0::2], 2, axis=1).T                          # [64, 4096]
    sinE = np.repeat(rf[:, 1::2], 2, axis=1).T
    sgn = np.where(np.arange(DH) % 2 == 0, -1.0, 1.0)[:, None]
    sinS = (sinE * sgn)
    cos2 = np.ascontiguousarray(np.concatenate([cosE, cosE], 0)).astype(_bf16)
    sin2 = np.ascontiguousarray(np.concatenate([sinS, sinS], 0)).astype(_bf16)

    pm = np.zeros((128, 128), np.float32)
    for d in range(128):
        pm[d ^ 1, d] = 1.0       # partner[d] = q[d^1]; lhsT = S (symmetric)
    perm = pm.astype(_bf16)
    ident = np.eye(128, dtype=np.float32).astype(_bf16)

    w3 = Wqkv.reshape(3, H, DH, HID)
    in_maps = []
    for c in range(NCORES):
        blocks = []
        for which in range(3):
            for hl in range(HPC):
                blocks.append(w3[which, 2 * c + hl])                    # [64, 1024]
        wq = np.concatenate(blocks, 0)                                  # [384, 1024]
        wqkvT = np.ascontiguousarray(wq.T).astype(_bf16)                # [1024, 384]
        in_maps.append({
            "xT": xf, "wqkvT": wqkvT, "woutT": woutT,
            "cos2": cos2, "sin2": sin2, "perm": perm, "ident": ident,
        })
    return in_maps


_CACHE = {}


def kernel(x, rope, Wqkv, Wout):
    from concourse.bass_utils import run_bass_kernel_spmd

    if "nc" not in _CACHE:
        _CACHE["nc"] = _build_graph()
    nc = _CACHE["nc"]
    in_maps = _host_inputs(np.asarray(x, np.float32), np.asarray(rope, np.float32),
                           np.asarray(Wqkv, np.float32), np.asarray(Wout, np.float32))
    res = run_bass_kernel_spmd(nc, in_maps, core_ids=list(range(NCORES)))
    parts = np.stack([np.asarray(res.results[i]["out"], np.float32)
                      for i in range(NCORES)])                          # [8, 512, 1024]
    # core j's rows [p*128:(p+1)*128] hold global tokens p*1024 + j*128 ..
    full = parts.reshape(NCORES, NPIECE, 128, HID).transpose(1, 0, 2, 3)
    return np.ascontiguousarray(full.reshape(T, HID)).reshape(B, N, HID)


# revision 28
# speedup vs baseline: 1.0834x; 1.0274x over previous
"""Distributed Bass kernel: fused multi-head attention block on 8 TRN2 NeuronCores.

Problem: x[2,2048,1024] -> QKV proj -> RoPE(q,k) -> softmax(q k^T/8) v -> out proj.

Sharding: tensor-parallel over heads. 16 heads / 8 cores = 2 heads per core.
Each core computes QKV for its 2 heads (full sequence), RoPE, attention, then
chunked AllToAlls (4 pieces of 1024 tokens, cc_dim=Free, pipelined under the
attention compute) convert head-sharding to token-sharding so the output
projection runs against the FULL Wout with no AllReduce. Token ownership is
interleaved per 128-token tile: within piece p core j owns tokens
[p*1024+j*128, p*1024+(j+1)*128). Host reassembles the 4x128-row tiles.

Schedule: QKV/RoPE for batch 0 runs first (chunks 0-3); then batch-1 QKV
chunks interleave with batch-0 attention chunks so the scalar engine's exp
stream starts ~50us earlier and the PE never idles long enough to re-throttle
(HAM). A dummy 2KB AllToAll during phase 1 absorbs the CC-stream first-op
warmup (~11.5us) off the critical path. Per-piece gathers are emitted two
chunks after the collective trigger so the gpsimd queue never parks on an
in-flight collective while attention still needs broadcasts.

Compute dtype bf16, f32 PSUM accumulation. Softmax skips the max-subtraction
(scores ~N(0,2), exp safe in f32) and folds the denominator into the PV matmul
via a ones-column in the per-head v table ([key,130] slots: vA|1|vB|1).
"""

import sys

for _p in ("/opt/trn_rl_repo", "/root/.axon_site/_ro/trn_rl_repo"):
    if _p not in sys.path:
        sys.path.append(_p)

import numpy as np
import ml_dtypes

B, N, HID = 2, 2048, 1024
H, DH = 16, 64
NCORES = 8
HPC = H // NCORES          # heads per core = 2
T = B * N                  # 4096 flattened tokens
TS = T // NCORES           # 512 tokens per core after AllToAll
EPC = HPC * DH             # 128 features per core
CH = 512                   # token chunk for QKV phase
NCH = T // CH              # 8 chunks
KT = 128                   # key tile
QC = 512                   # query chunk in attention
NPIECE = 4                 # a2a pieces (1024 tokens each)
PT = T // NPIECE           # 1024 tokens per piece
VS = 2 * (DH + 1)          # 130-wide v-table slot: [vA(64) | 1 | vB(64) | 1]

_bf16 = ml_dtypes.bfloat16


def _build_graph():
    import concourse.bass as bass
    import concourse.mybir as mybir
    import concourse.tile as tile
    from concourse import bacc

    f32 = mybir.dt.float32
    bf16 = mybir.dt.bfloat16

    nc = bacc.Bacc("TRN2", target_bir_lowering=False, debug=False, num_devices=NCORES)

    xT_e = nc.declare_dram_parameter("xT", [HID, T], bf16, isOutput=False)
    wqkvT_e = nc.declare_dram_parameter("wqkvT", [HID, 3 * EPC], bf16, isOutput=False)
    woutT_e = nc.declare_dram_parameter("woutT", [HID, HID], bf16, isOutput=False)
    cos2_e = nc.declare_dram_parameter("cos2", [2 * DH, T], bf16, isOutput=False)
    sin2_e = nc.declare_dram_parameter("sin2", [2 * DH, T], bf16, isOutput=False)
    perm_e = nc.declare_dram_parameter("perm", [128, 128], bf16, isOutput=False)
    ident_e = nc.declare_dram_parameter("ident", [128, 128], bf16, isOutput=False)
    out_e = nc.declare_dram_parameter("out", [TS, HID], f32, isOutput=True)

    with tile.TileContext(nc) as tc:
        with (
            tc.tile_pool(name="const", bufs=1) as cpool,
            tc.tile_pool(name="work", bufs=1) as wpool,
            tc.tile_pool(name="stream", bufs=4) as spool,
            tc.tile_pool(name="psum", bufs=2, space="PSUM") as pspool,
            tc.tile_pool(name="dram", bufs=1, space="DRAM") as dpool,
        ):
            # ---- constants / weights ----
            wqkvT = cpool.tile([128, 8 * 3 * EPC], bf16)       # 8 k-tiles side by side
            for kt in range(8):
                nc.gpsimd.dma_start(
                    wqkvT[:, kt * 3 * EPC:(kt + 1) * 3 * EPC],
                    wqkvT_e[kt * 128:(kt + 1) * 128, :],
                )
            perm = cpool.tile([128, 128], bf16)
            nc.gpsimd.dma_start(perm[:, :], perm_e[:, :])
            ident = cpool.tile([128, 128], bf16)
            nc.gpsimd.dma_start(ident[:, :], ident_e[:, :])
            woutT = cpool.tile([128, 8 * HID], bf16)
            cos2 = cpool.tile([128, T], bf16)
            sin2 = cpool.tile([128, T], bf16)

            # ---- persistent working tensors ----
            q_sb = wpool.tile([128, T], bf16)      # raw q (rope intermediate)
            k_sb = wpool.tile([128, T], bf16)      # becomes roped k
            qzA = wpool.tile([128, T], bf16)       # roped qA rows 0-63, 0 below
            qzB = wpool.tile([128, T], bf16)       # roped qB rows 64-127, 0 above
            vtab = wpool.tile([128, 32 * VS], bf16)  # [key, vA|1|vB|1] per slot
            ovT = wpool.tile([128, T], bf16)       # attention out ^T

            vt3 = vtab.rearrange("p (s c) -> p s c", c=VS)
            nc.vector.memset(qzA[DH:128, :], 0.0)
            nc.vector.memset(qzB[0:DH, :], 0.0)
            nc.vector.memset(vt3[:, :, DH:DH + 1], 1.0)
            nc.vector.memset(vt3[:, :, 2 * DH + 1:2 * DH + 2], 1.0)

            # ================= Phase 1 chunk: QKV + RoPE + v-transpose ============
            def phase1_chunk(c):
                xs = []
                for kt in range(8):
                    xt = spool.tile([128, CH], bf16, tag="xs", bufs=10)
                    nc.gpsimd.dma_start(
                        xt[:, :], xT_e[kt * 128:(kt + 1) * 128, c * CH:(c + 1) * CH]
                    )
                    xs.append(xt)
                if c < 2:
                    # rope factors for batch-half c arrive under chunk-c compute
                    hs = slice(c * (T // 2), (c + 1) * (T // 2))
                    nc.gpsimd.dma_start(sin2[:, hs], sin2_e[:, hs])
                    nc.gpsimd.dma_start(cos2[:, hs], cos2_e[:, hs])
                sl = slice(c * CH, (c + 1) * CH)
                vTc = spool.tile([128, CH], bf16, tag="vTc", bufs=2)
                for which, dest, dsl in ((0, q_sb, sl), (1, k_sb, sl),
                                         (2, vTc, slice(0, CH))):
                    ps = pspool.tile([128, CH], f32, tag="mm", bufs=2)
                    for kt in range(8):
                        nc.tensor.matmul(
                            ps[:, :],
                            wqkvT[:, kt * 3 * EPC + which * EPC:
                                  kt * 3 * EPC + (which + 1) * EPC],
                            xs[kt][:, :],
                            start=(kt == 0),
                            stop=(kt == 7),
                        )
                    nc.vector.tensor_copy(dest[:, dsl], ps[:, :])
                # transpose v chunk into the 130-wide per-slot v tables
                for tt in range(CH // 128):
                    slot = c * (CH // 128) + tt
                    tsl = slice(tt * 128, (tt + 1) * 128)
                    tp = pspool.tile([128, 128], bf16, tag="sc", bufs=2)
                    nc.tensor.transpose(tp[:, :], vTc[:, tsl], ident[:, :])
                    nc.vector.tensor_copy(vt3[:, slot, 0:DH], tp[:, 0:DH])
                    nc.vector.tensor_copy(
                        vt3[:, slot, DH + 1:2 * DH + 1], tp[:, DH:2 * DH]
                    )
                # RoPE on q and k: t = P@x * sin2 ; rot = x*cos2 + t
                for srd, dests in ((q_sb, ((qzA, 0, DH), (qzB, DH, 128))),
                                   (k_sb, ((k_sb, 0, 128),))):
                    pps = pspool.tile([128, CH], f32, tag="mm", bufs=2)
                    nc.tensor.matmul(
                        pps[:, :], perm[:, :], srd[:, sl],
                        start=True, stop=True,
                    )
                    tmp = spool.tile([128, CH], bf16, tag="ropetmp", bufs=2)
                    nc.vector.tensor_mul(tmp[:, :], pps[:, :], sin2[:, sl])
                    nc.vector.tensor_mul(srd[:, sl], srd[:, sl], cos2[:, sl])
                    for dst, p0, p1 in dests:
                        nc.vector.tensor_add(
                            dst[p0:p1, sl], srd[p0:p1, sl], tmp[p0:p1, :]
                        )

            # ================= Attention machinery ================================
            NKT = N // KT                      # 16 key tiles per chunk
            qzs = (qzA, qzB)

            def emit_pv_pair(st, pair):
                (b, qc, ops, expTs) = st
                for h in range(HPC):
                    for kt in (2 * pair, 2 * pair + 1):
                        slot = b * (N // 128) + kt
                        nc.tensor.matmul(
                            ops[0:DH + 1, h * QC:(h + 1) * QC],
                            vtab[:, slot * VS + h * (DH + 1):
                                 slot * VS + (h + 1) * (DH + 1)],
                            expTs[h][:, kt * QC:(kt + 1) * QC],
                            start=(kt == 0),
                            stop=(kt == NKT - 1),
                        )

            def emit_normalize(st):
                (b, qc, ops, expTs) = st
                q0 = b * N + qc * QC
                for h in range(HPC):
                    hr = h * DH
                    hc = h * QC
                    den = spool.tile([1, QC], f32, tag="den", bufs=2)
                    nc.vector.tensor_copy(den[0:1, :], ops[DH:DH + 1, hc:hc + QC])
                    rec = spool.tile([1, QC], f32, tag="rec", bufs=2)
                    nc.vector.reciprocal_approx_fast(rec[0:1, :], den[0:1, :])
                    bcs = spool.tile([64, QC], f32, tag="bcs", bufs=2)
                    nc.gpsimd.partition_broadcast(bcs[:, :], rec[0:1, :])
                    nc.vector.tensor_mul(
                        ovT[hr:hr + DH, q0:q0 + QC], ops[0:DH, hc:hc + QC], bcs[:, :]
                    )

            a2a_in = [dpool.tile([NCORES * 128, PT // NCORES], bf16,
                                 name=f"a2a_in{p}") for p in range(NPIECE)]
            a2a_out = [dpool.tile([NCORES * 128, PT // NCORES], bf16,
                                  name=f"a2a_out{p}") for p in range(NPIECE)]

            def emit_comm(p):
                # stage my features for every peer's token tile of piece p,
                # then AllToAll (input split along dim 0, one block per peer)
                for j in range(NCORES):
                    c0 = p * PT + j * 128
                    nc.gpsimd.dma_start(
                        a2a_in[p][j * 128:(j + 1) * 128, :],
                        ovT[:, c0:c0 + 128],
                    )
                nc.gpsimd.collective_compute(
                    "AllToAll",
                    mybir.AluOpType.bypass,
                    ins=[a2a_in[p].opt()],
                    outs=[a2a_out[p].opt()],
                    replica_groups=[list(range(NCORES))],
                )

            def emit_outproj(p):
                gT = spool.tile([128, NCORES * 128], bf16, tag="gT", bufs=2)
                for s in range(NCORES):
                    nc.gpsimd.dma_start(
                        gT[:, s * 128:(s + 1) * 128],
                        a2a_out[p][s * 128:(s + 1) * 128, :],
                    )
                for nn in range(HID // 512):
                    odps = pspool.tile([128, 512], f32, tag="mm", bufs=2)
                    for s in range(8):
                        nc.tensor.matmul(
                            odps[:, :],
                            gT[:, s * 128:(s + 1) * 128],
                            woutT[:, s * HID + nn * 512:s * HID + (nn + 1) * 512],
                            start=(s == 0),
                            stop=(s == 7),
                        )
                    osb = spool.tile([128, 512], f32, tag="osb", bufs=2)
                    nc.vector.tensor_copy(osb[:, :], odps[:, :])
                    nc.gpsimd.dma_start(
                        out_e[p * 128:(p + 1) * 128, nn * 512:(nn + 1) * 512],
                        osb[:, :],
                    )

            st = {"pending": None}

            def attn_chunk(ci):
                b, qc = divmod(ci, N // QC)
                q0 = b * N + qc * QC
                expTs = (spool.tile([128, NKT * QC], bf16, name="expTA",
                                    tag="expTA", bufs=2),
                         spool.tile([128, NKT * QC], bf16, name="expTB",
                                    tag="expTB", bufs=2))
                for pair in range(NKT // 2):
                    for h in range(HPC):
                        sps = pspool.tile([128, 2 * QC], f32, tag="sc", bufs=2)
                        for half in range(2):
                            k0 = b * N + (2 * pair + half) * KT
                            nc.tensor.matmul(
                                sps[:, half * QC:(half + 1) * QC],
                                k_sb[:, k0:k0 + KT],
                                qzs[h][:, q0:q0 + QC],
                                start=True, stop=True,
                            )
                        nc.scalar.activation(
                            expTs[h][:, 2 * pair * QC:(2 * pair + 2) * QC],
                            sps[:, :],
                            mybir.ActivationFunctionType.Exp,
                            scale=DH ** -0.5,
                        )
                    if st["pending"] is not None:
                        emit_pv_pair(st["pending"], pair)
                        if pair == NKT // 2 - 1:
                            emit_normalize(st["pending"])
                ops = pspool.tile([128, 2 * QC], f32, tag="pv", bufs=1)
                st["pending"] = (b, qc, ops, expTs)
                if ci in (2, 4, 6):
                    emit_comm((ci - 2) // 2)
                if ci in (5, 7):
                    # one extra chunk of slack: the early collectives ramp
                    # slowly (~9-17 GB/s) and outrun a single-chunk window
                    emit_outproj((ci - 5) // 2)

            # ================= Top-level schedule ================================
            # CC-stream warmup: tiny collective with no dependencies, absorbed
            # under phase-1 compute (its payload is never read).
            ccw_in = dpool.tile([NCORES, 128], bf16, name="ccw_in")
            ccw_out = dpool.tile([NCORES, 128], bf16, name="ccw_out")

            for c in range(4):
                phase1_chunk(c)
                if c == 0:
                    nc.gpsimd.collective_compute(
                        "AllToAll",
                        mybir.AluOpType.bypass,
                        ins=[ccw_in.opt()],
                        outs=[ccw_out.opt()],
                        replica_groups=[list(range(NCORES))],
                    )
            for kt in range(8):
                nc.gpsimd.dma_start(
                    woutT[:, kt * HID:(kt + 1) * HID],
                    woutT_e[kt * 128:(kt + 1) * 128, :],
                )
            for i in range(4):
                phase1_chunk(4 + i)
                attn_chunk(i)
            for ci in range(4, 8):
                attn_chunk(ci)
            for pair in range(NKT // 2):
                emit_pv_pair(st["pending"], pair)
            emit_normalize(st["pending"])
            emit_outproj(2)
            emit_comm(3)
            emit_outproj(3)

    nc.finalize()
    return nc


def _host_inputs(x, rope, Wqkv, Wout):
    """Build the 8 per-core input maps with host-side layout prep."""
    xf = np.ascontiguousarray(x.reshape(T, HID).T).astype(_bf16)        # [1024, 4096]
    woutT = np.ascontiguousarray(Wout.T).astype(_bf16)                  # [1024, 1024]

    rf = rope.reshape(T, DH)                                            # [4096, 64]
    cosE = np.repeat(rf[:, 0::2], 2, axis=1).T                          # [64, 4096]
    sinE = np.repeat(rf[:, 1::2], 2, axis=1).T
    sgn = np.where(np.arange(DH) % 2 == 0, -1.0, 1.0)[:, None]
    sinS = (sinE * sgn)
    cos2 = np.ascontiguousarray(np.concatenate([cosE, cosE], 0)).astype(_bf16)
    sin2 = np.ascontiguousarray(np.concatenate([sinS, sinS], 0)).astype(_bf16)

    pm = np.zeros((128, 128), np.float32)
    for d in range(128):
        pm[d ^ 1, d] = 1.0       # partner[d] = q[d^1]; lhsT = S (symmetric)
    perm = pm.astype(_bf16)
    ident = np.eye(128, dtype=np.float32).astype(_bf16)

    w3 = Wqkv.reshape(3, H, DH, HID)
    in_maps = []
    for c in range(NCORES):
        blocks = []
        for which in range(3):
            for hl in range(HPC):
                blocks.append(w3[which, 2 * c + hl])                    # [64, 1024]
        wq = np.concatenate(blocks, 0)                                  # [384, 1024]
        wqkvT = np.ascontiguousarray(wq.T).astype(_bf16)                # [1024, 384]
        in_maps.append({
            "xT": xf, "wqkvT": wqkvT, "woutT": woutT,
            "cos2": cos2, "sin2": sin2, "perm": perm, "ident": ident,
        })
    return in_maps


_CACHE = {}


def kernel(x, rope, Wqkv, Wout):
    from concourse.bass_utils import run_bass_kernel_spmd

    if "nc" not in _CACHE:
        _CACHE["nc"] = _build_graph()
    nc = _CACHE["nc"]
    in_maps = _host_inputs(np.asarray(x, np.float32), np.asarray(rope, np.float32),
                           np.asarray(Wqkv, np.float32), np.asarray(Wout, np.float32))
    res = run_bass_kernel_spmd(nc, in_maps, core_ids=list(range(NCORES)))
    parts = np.stack([np.asarray(res.results[i]["out"], np.float32)
                      for i in range(NCORES)])                          # [8, 512, 1024]
    # core j's rows [p*128:(p+1)*128] hold global tokens p*1024 + j*128 ..
    full = parts.reshape(NCORES, NPIECE, 128, HID).transpose(1, 0, 2, 3)
    return np.ascontiguousarray(full.reshape(T, HID)).reshape(B, N, HID)
